# revision 24
# baseline (speedup 1.0000x reference)
"""Trainium2 Bass kernel for nn_LowRankInterpLinearOperator2d.

out[b,o,h,w] = sum_r vr[b,r]*k2i[r,o,h,w] + sum_i conv_w[o,i]*v[b,i,h,w]
               + conv_b[o] + bias[o]

Factorization (no k1i/k2i materialization):
  G[pq, hw]   = wy[h,p] * wx[w,q]                      (16 x 16384, host)
  vproj[i,pq] = sum_hw v[i,hw] * G[pq,hw]              (PE transposes + matmuls)
  vr[r]       = sum_{i,pq} k1[r,i,pq]/HW * vproj[i,pq] (tiny matmuls)
  t2[o,pq]    = sum_r vr[r] * k2[r,o,pq]               (tiny matmuls)
  out[o,hw]   = conv_w @ v  +  t2 @ G  + cb            (PSUM-accumulated)

Key engineering (DMA cost on trn2 is per-partition bytes; PE matmul
operand base partitions must be 0/32/64):
- all-bf16 dataflow: v in and out shipped bf16 (halves both DMA legs)
- every DMA is a full-width 128-partition layout; G for the t2G pass is
  packed [128, 2048] as 8 groups of 16 pq-rows at offsets 64k+16s; the
  K=64 matmul uses an lhsT (t2B) holding t2^T at row offset 16s and
  zeros elsewhere, so each group's rows are selected exactly
- t2B built by 8 window-transposes over a zero-padded staging buffer;
  vr replicated to partition quadrants {0,32} so the packed k2 slices
  share operand bases
- DMA order: one leading transfer carrying ident+convwT+gtbf, then v
  (tail as two 1024-col pieces), then the small-chain/main params (k1/k2
  packed in one transfer) in consumption order, G split in two pieces;
  params share DMAs to amortize the ~500ns per-DMA descriptor floor
- conv for chunks 0-2 pre-run into the otherwise-idle output PSUM banks
  during v arrival; conv for chunks 3-6 runs in the small chain's
  cross-engine wait gaps, into the freed transpose banks (f32 bitcast)
- PE un-stalled: 4 transpose PSUM slots, vproj trails transposes by 3
  chunks into a persistent v^T buffer, PSUM->SBUF copies split DVE/Act
  weighted 5:3, output staged in 1024-col groups

Sharding: data-parallel over batch B=8, one batch per NeuronCore.
"""

import numpy as np
import ml_dtypes

import concourse.bass as bass
import concourse.mybir as mybir
from concourse.bass_utils import run_bass_kernel_spmd

F32 = mybir.dt.float32
BF16 = mybir.dt.bfloat16
BF16_NP = ml_dtypes.bfloat16

B, Cin, Cout, H, W = 8, 128, 128, 128, 128
RANK, R4 = 32, 4
PQ = R4 * R4  # 16
HW = H * W  # 16384
N_CORES = 8
CHUNK = 512
NCHUNK = HW // CHUNK  # 32
SUB = 128
NSUB = HW // SUB  # 128
SLAB = 2048
NSLAB = HW // SLAB  # 8


def _interp_matrix(n_out, r):
    # match reference.interp_matrix bit-for-bit (float32 arithmetic)
    t = ((np.arange(n_out, dtype=np.float32) + np.float32(0.5))
         / np.float32(n_out) * np.float32(r - 1)).astype(np.float32)
    i0 = np.clip(np.floor(t), 0, r - 2).astype(np.int32)
    frac = (t - i0.astype(np.float32)).astype(np.float32)
    w = np.zeros((n_out, r), np.float32)
    w[np.arange(n_out), i0] = np.float32(1.0) - frac
    w[np.arange(n_out), i0 + 1] = frac
    return w


def _build_nc():
    from contextlib import ExitStack
    nc = bass.Bass()
    v_d = nc.declare_dram_parameter("v", [Cin, HW], BF16, isOutput=False)
    pbg_d = nc.declare_dram_parameter("pbg", [128, 256 + NSUB * PQ], BF16, isOutput=False)
    gp_d = nc.declare_dram_parameter("gp", [128, 2048], BF16, isOutput=False)
    kp_d = nc.declare_dram_parameter("kp", [128, 1540], BF16, isOutput=False)
    out_d = nc.declare_dram_parameter("out", [Cout, HW], BF16, isOutput=True)

    es = ExitStack()
    with es:
        v_sb = es.enter_context(nc.sbuf_tensor("v_sb", [Cin, HW], BF16))
        pbg = es.enter_context(nc.sbuf_tensor("pbg_sb", [128, 256 + NSUB * PQ], BF16))
        ident = pbg[:, 0:128]
        convwT = pbg[:, 128:256]
        gp = es.enter_context(nc.sbuf_tensor("gp_sb", [128, 2048], BF16))
        kp = es.enter_context(nc.sbuf_tensor("kp_sb", [128, 1540], BF16))
        k1p = kp[:, 0:512]
        k2p = kp[0:64, 512:1536]
        cb = kp[:, 1536:1538].bitcast(F32)  # [128, 1] f32 bit-pattern
        vtall = es.enter_context(nc.sbuf_tensor("vtall_sb", [128, HW], BF16))
        osb = [es.enter_context(nc.sbuf_tensor(f"osb{i}", [Cout, 2 * CHUNK], BF16)) for i in range(4)]
        vproj_sb = es.enter_context(nc.sbuf_tensor("vproj_sb", [Cin, PQ], BF16))
        vr_rep = es.enter_context(nc.sbuf_tensor("vr_rep_sb", [64, 1], BF16))
        t2z = es.enter_context(nc.sbuf_tensor("t2z_sb", [Cout, 192], BF16))
        t2B = es.enter_context(nc.sbuf_tensor("t2B_sb", [128, 512], BF16))

        # full-bank transpose buffers: bf16 [128,1024] each; after phase A
        # each bank is reused (via f32 bitcast) as a conv accumulator for
        # chunks 3..6, whose conv runs during the small chain's DVE hops
        tpsb = [es.enter_context(nc.psum_tensor(f"tps{i}", [128, 2 * CHUNK], BF16)) for i in range(4)]
        tps = [t[:, 0:CHUNK] for t in tpsb]
        tpsF = [t[:].bitcast(F32) for t in tpsb]  # [128, 512] f32 views
        ops = [es.enter_context(nc.psum_tensor(f"ops{i}", [Cout, CHUNK], F32)) for i in range(3)]
        small_ps = es.enter_context(nc.psum_tensor("small_ps", [128, CHUNK], F32))
        vr_ps = small_ps[0:64, 0:1]
        vproj_acc = small_ps[0:Cin, 208:208 + PQ]
        t2_ps = small_ps[0:Cout, 64:64 + PQ]
        t2B_ps = small_ps[0:128, 256:512].bitcast(BF16)  # [128, 512] bf16

        sem_pa = es.enter_context(nc.semaphore("sem_pa"))  # ident
        sem_pb = es.enter_context(nc.semaphore("sem_pb"))  # k1p+k2p
        sem_pc = es.enter_context(nc.semaphore("sem_pc"))  # cb+gp1
        sem_gp2 = es.enter_context(nc.semaphore("sem_gp2"))
        # one semaphore per v DMA: waits are always at the full value 16,
        # sound under any DMA completion ordering
        sem_s = [es.enter_context(nc.semaphore(f"sem_s{i}")) for i in range(NSLAB - 1)]
        sem_t = [es.enter_context(nc.semaphore(f"sem_t{i}")) for i in range(2)]
        sem_pe_t = es.enter_context(nc.semaphore("sem_pe_t"))
        sem_vtd = es.enter_context(nc.semaphore("sem_vtd"))  # DVE vt copies (even c)
        sem_vta = es.enter_context(nc.semaphore("sem_vta"))  # Act vt copies (odd c)
        sem_pe_vp = es.enter_context(nc.semaphore("sem_pe_vp"))
        sem_dve_sm = es.enter_context(nc.semaphore("sem_dve_sm"))
        sem_pe_sm = es.enter_context(nc.semaphore("sem_pe_sm"))
        sem_pe_main = es.enter_context(nc.semaphore("sem_pe_main"))
        sem_cpd = es.enter_context(nc.semaphore("sem_cpd"))  # DVE out copies (even c)
        sem_cpa = es.enter_context(nc.semaphore("sem_cpa"))  # Act out copies (odd c)
        # per-osb-slot out-DMA completion sems (always waited at running total)
        sem_ob = [es.enter_context(nc.semaphore(f"sem_ob{i}")) for i in range(4)]

        block = es.enter_context(nc.Block())

        # sem covering arrival of v chunk c
        def v_sem(c):
            if c < (NSLAB - 1) * 4:
                return sem_s[c // 4]
            return sem_t[(c - (NSLAB - 1) * 4) // 2]

        # weighted DVE/Act copier maps (DVE is faster; ~3:2 split)
        VT_OF = [1 if c % 8 in (1, 4, 6) else 0 for c in range(NCHUNK)]
        OC_OF = [1 if c % 8 in (1, 4, 6) else 0 for c in range(NCHUNK)]

        def vt_done(c):
            return (sem_vtd if VT_OF[c] == 0 else sem_vta), \
                len([d for d in range(c + 1) if VT_OF[d] == VT_OF[c]])

        def out_done(c):
            return (sem_cpd if OC_OF[c] == 0 else sem_cpa), \
                len([d for d in range(c + 1) if OC_OF[d] == OC_OF[c]])

        # PSUM accumulator for output chunk c: 0-2 in ops, 3-6 in the
        # repurposed transpose banks, 7+ rotating over ops
        def main_dst(c):
            if c < 3:
                return ops[c][:]
            if c < 7:
                return tpsF[c - 3]
            return ops[(c - 7) % 3][:]

        @block.sync
        def _(sync):
            sync.dma_start(out=pbg[:], in_=pbg_d[:]).then_inc(sem_pa, 16)
            sync.dma_start(
                out=v_sb[:, 0:SLAB], in_=v_d[:, 0:SLAB]).then_inc(sem_s[0], 16)
            for s in range(1, NSLAB - 1):
                sync.dma_start(
                    out=v_sb[:, s * SLAB:(s + 1) * SLAB],
                    in_=v_d[:, s * SLAB:(s + 1) * SLAB],
                ).then_inc(sem_s[s], 16)
            for j in range(2):
                lo = (NSLAB - 1) * SLAB + j * (SLAB // 2)
                sync.dma_start(
                    out=v_sb[:, lo:lo + SLAB // 2],
                    in_=v_d[:, lo:lo + SLAB // 2],
                ).then_inc(sem_t[j], 16)
            sync.dma_start(out=kp[:], in_=kp_d[:]).then_inc(sem_pb, 16)
            sync.dma_start(out=gp[:, 0:CHUNK], in_=gp_d[:, 0:CHUNK]).then_inc(sem_pc, 16)
            sync.dma_start(out=gp[:, CHUNK:2048], in_=gp_d[:, CHUNK:2048]).then_inc(sem_gp2, 16)
            for g in range(NCHUNK // 2 - 1):
                for c in (2 * g, 2 * g + 1):
                    s, n = out_done(c)
                    sync.wait_ge(s, n)
                sync.dma_start(
                    out=out_d[:, 2 * g * CHUNK:(2 * g + 2) * CHUNK],
                    in_=osb[g % 4][:],
                ).then_inc(sem_ob[g % 4], 16)
            gl = NCHUNK // 2 - 1
            for j, c in enumerate((NCHUNK - 2, NCHUNK - 1)):
                s, n = out_done(c)
                sync.wait_ge(s, n)
                sync.dma_start(
                    out=out_d[:, c * CHUNK:(c + 1) * CHUNK],
                    in_=osb[gl % 4][:, j * CHUNK:(j + 1) * CHUNK],
                ).then_inc(sem_ob[gl % 4], 16)
            for k in range(4):
                uses = len([g for g in range(NCHUNK // 2) if g % 4 == k])
                if k == (NCHUNK // 2 - 1) % 4:
                    uses += 1  # final slot drains via two single-chunk DMAs
                sync.wait_ge(sem_ob[k], 16 * uses)

        @block.tensor
        def _(tensor):
            tensor.wait_ge(sem_pa, 16)  # ident

            def trans(c):
                tensor.wait_ge(v_sem(c), 16)
                if c >= 4:
                    s, n = vt_done(c - 4)
                    tensor.wait_ge(s, n)
                ins = None
                for k in range(4):
                    ins = tensor.transpose(
                        tps[c % 4][:, k * 128:(k + 1) * 128],
                        v_sb[:, c * CHUNK + k * 128:c * CHUNK + (k + 1) * 128],
                        ident,
                    )
                ins.then_inc(sem_pe_t, 1)

            def vproj_mm(c):
                s, n = vt_done(c)
                tensor.wait_ge(s, n)
                ins = None
                for k in range(4):
                    h = 4 * c + k
                    ins = tensor.matmul(
                        vproj_acc,
                        lhsT=vtall[:, h * 128:(h + 1) * 128],
                        rhs=pbg[:, 256 + h * PQ:256 + (h + 1) * PQ],
                        start=(h == 0),
                        stop=(h == NSUB - 1),
                        skip_group_check=True,
                    )
                ins.then_inc(sem_pe_vp, 1)

            gt_gate = [False]
            for c in range(NCHUNK):
                trans(c)
                if c < 3:
                    tensor.matmul(
                        ops[c][:],
                        lhsT=convwT,
                        rhs=v_sb[:, c * CHUNK:(c + 1) * CHUNK],
                        start=True,
                        stop=False,
                        skip_group_check=True,
                    )
                if c >= 3:
                    if not gt_gate[0]:
                        gt_gate[0] = True
                    vproj_mm(c - 3)
            for c in range(NCHUNK - 3, NCHUNK):
                vproj_mm(c)

            def conv_gap(c):
                tensor.matmul(
                    tpsF[c - 3],
                    lhsT=convwT,
                    rhs=v_sb[:, c * CHUNK:(c + 1) * CHUNK],
                    start=True,
                    stop=False,
                    skip_group_check=True,
                )

            # small chain: vr -> t2 -> t2B, with conv(3..6) filling the
            # PE idle gaps between the chain's cross-engine hops
            conv_gap(3)
            tensor.wait_ge(sem_pb, 16)  # kp (k1p+k2pp)
            tensor.wait_ge(sem_dve_sm, 1)
            ins = None
            for g in range(2):
                for pq in range(PQ):
                    ins = tensor.matmul(
                        vr_ps[g * 32:(g + 1) * 32, 0:1],
                        lhsT=kp[:, pq * RANK:(pq + 1) * RANK],
                        rhs=vproj_sb[:, pq:pq + 1],
                        start=(pq == 0),
                        stop=(pq == PQ - 1),
                        skip_group_check=True,
                    )
            ins.then_inc(sem_pe_sm, 1)
            conv_gap(4)
            tensor.wait_ge(sem_dve_sm, 2)
            ins = None
            for pq in range(PQ):
                g, m0 = pq // 8, (pq % 8) * 128
                ins = tensor.matmul(
                    t2_ps[:, pq:pq + 1],
                    lhsT=kp[g * 32:(g + 1) * 32, 512 + m0:512 + m0 + 128],
                    rhs=vr_rep[g * 32:(g + 1) * 32, 0:1],
                    start=True,
                    stop=True,
                    skip_group_check=True,
                )
            ins.then_inc(sem_pe_sm, 1)
            conv_gap(5)
            tensor.wait_ge(sem_dve_sm, 3)
            for k in range(2):
                ins = None
                for s in range(4):
                    ins = tensor.transpose(
                        t2B_ps[64 * k:64 * k + 64, 128 * s:128 * (s + 1)],
                        t2z[:, 64 - 16 * s:128 - 16 * s],
                        ident,
                    )
                ins.then_inc(sem_pe_sm, 1)

            # main loop: out = convwT.T @ v + t2T.T @ G (+cb on copy-out)
            conv_gap(6)
            tensor.wait_ge(sem_pc, 16)  # gp1
            tensor.wait_ge(sem_dve_sm, 4)
            gp2_gate = [False]
            t2b1_gate = [False]
            for c in range(NCHUNK):
                if not t2b1_gate[0] and c % 8 >= 4:
                    tensor.wait_ge(sem_dve_sm, 5)
                    t2b1_gate[0] = True
                if not gp2_gate[0] and c >= 8:
                    tensor.wait_ge(sem_gp2, 16)
                    gp2_gate[0] = True
                if c >= 7:
                    s, n = out_done(c - 7 if c < 10 else c - 3)
                    tensor.wait_ge(s, n)
                    tensor.matmul(
                        main_dst(c),
                        lhsT=convwT,
                        rhs=v_sb[:, c * CHUNK:(c + 1) * CHUNK],
                        start=True,
                        stop=False,
                        skip_group_check=True,
                    )
                k8, s8, loc = (c % 8) // 4, (c % 8) % 4, c // 8
                tensor.matmul(
                    main_dst(c),
                    lhsT=t2B[64 * k8:64 * k8 + 64, 128 * s8:128 * (s8 + 1)],
                    rhs=gp[64 * k8:64 * k8 + 64, loc * CHUNK:(loc + 1) * CHUNK],
                    start=False,
                    stop=True,
                    skip_group_check=True,
                ).then_inc(sem_pe_main, 1)

        @block.vector
        def _(vector):
            # zero t2z staging pad once (window transposes read zeros there)
            vector.memset(t2z[:], 0.0)
            # phase A: vt copies (weighted share)
            for c in range(NCHUNK):
                if VT_OF[c] != 0:
                    continue
                vector.wait_ge(sem_pe_t, c + 1)
                vector.tensor_copy(
                    vtall[:, c * CHUNK:(c + 1) * CHUNK], tps[c % 4][:]
                ).then_inc(sem_vtd, 1)

            # small chain copies
            vector.wait_ge(sem_pe_vp, NCHUNK)
            vector.tensor_copy(vproj_sb[:], vproj_acc).then_inc(sem_dve_sm, 1)
            vector.wait_ge(sem_pe_sm, 1)
            vector.tensor_copy(vr_rep[:], vr_ps).then_inc(sem_dve_sm, 1)
            vector.wait_ge(sem_pe_sm, 2)
            vector.tensor_copy(t2z[:, 64:64 + PQ], t2_ps).then_inc(sem_dve_sm, 1)
            vector.wait_ge(sem_pe_sm, 3)
            vector.tensor_copy(t2B[0:64, :], t2B_ps[0:64, :]).then_inc(sem_dve_sm, 1)
            vector.wait_ge(sem_pe_sm, 4)
            vector.tensor_copy(t2B[64:128, :], t2B_ps[64:128, :]).then_inc(sem_dve_sm, 1)

            # main: out copies (+cb) for even chunks
            for c in range(NCHUNK):
                if OC_OF[c] != 0:
                    continue
                vector.wait_ge(sem_pe_main, c + 1)
                g = c // 2
                if g >= 4:
                    vector.wait_ge(sem_ob[g % 4], 16 * (g // 4))
                vector.tensor_scalar_add(
                    osb[g % 4][:, (c % 2) * CHUNK:(c % 2 + 1) * CHUNK],
                    main_dst(c), cb
                ).then_inc(sem_cpd, 1)

        @block.scalar
        def _(scalar):
            # phase A: vt copies (weighted share)
            for c in range(NCHUNK):
                if VT_OF[c] != 1:
                    continue
                scalar.wait_ge(sem_pe_t, c + 1)
                scalar.activation(
                    vtall[:, c * CHUNK:(c + 1) * CHUNK], tps[c % 4][:],
                    mybir.ActivationFunctionType.Identity,
                ).then_inc(sem_vta, 1)

            # main: out copies (+cb) for odd chunks
            scalar.wait_ge(sem_pb, 16)  # cb (in kp)
            for c in range(NCHUNK):
                if OC_OF[c] != 1:
                    continue
                scalar.wait_ge(sem_pe_main, c + 1)
                g = c // 2
                if g >= 4:
                    scalar.wait_ge(sem_ob[g % 4], 16 * (g // 4))
                scalar.activation(
                    osb[g % 4][:, (c % 2) * CHUNK:(c % 2 + 1) * CHUNK],
                    main_dst(c),
                    mybir.ActivationFunctionType.Identity,
                    bias=cb,
                ).then_inc(sem_cpa, 1)

    nc.finalize()
    return nc


_NC_CACHE = None


def _get_nc():
    global _NC_CACHE
    if _NC_CACHE is None:
        _NC_CACHE = _build_nc()
    return _NC_CACHE


def _make_in_maps(v, k1, k2, conv_w, conv_b, bias):
    wy = _interp_matrix(H, R4)  # (H, 4)
    wx = _interp_matrix(W, R4)  # (W, 4)
    # G[p*4+q, h*W+w] = wy[h,p]*wx[w,q]
    G = np.einsum("hp,wq->pqhw", wy, wx).reshape(PQ, HW).astype(np.float32)
    # gp[64k+16s+pq, L*512+m] = G[pq, c*512+m] for c = 8L + 4k + s
    gp = np.zeros((128, 2048), np.float32)
    for c in range(NCHUNK):
        k8, s8, L = (c % 8) // 4, (c % 8) % 4, c // 8
        r0 = 64 * k8 + 16 * s8
        gp[r0:r0 + PQ, L * CHUNK:(L + 1) * CHUNK] = G[:, c * CHUNK:(c + 1) * CHUNK]
    gp = gp.astype(BF16_NP)
    # kp = [ k1p | k2pp | cb-f32-bitpattern (2 bf16 cols) ]
    kp = np.zeros((128, 1540), np.float32)
    kp[:, 0:512] = (
        (k1.reshape(RANK, Cin, PQ) / np.float32(HW)).transpose(1, 2, 0)
    ).reshape(Cin, PQ * RANK)
    # k2pp[g*32+r, m] = k2p[r, g*1024+m] where k2p[r, pq*128+o] = k2[r,o,p,q]
    k2p_flat = np.ascontiguousarray(
        k2.reshape(RANK, Cout, PQ).transpose(0, 2, 1)
    ).reshape(RANK, PQ * Cout)
    kp[0:64, 512:1536] = (
        k2p_flat.reshape(RANK, 2, 1024).transpose(1, 0, 2).reshape(64, 1024)
    )
    kp = kp.astype(BF16_NP)
    cbv = (conv_b.reshape(Cout) + bias.reshape(Cout)).astype(np.float32)
    kp.view(np.uint16)[:, 1536:1538] = cbv.view(np.uint16).reshape(Cout, 2)
    kp = np.ascontiguousarray(kp)
    pbg = np.zeros((128, 256 + NSUB * PQ), np.float32)
    pbg[:, 0:128] = np.eye(128, dtype=np.float32)
    pbg[:, 128:256] = conv_w.T
    pbg[:, 256:] = np.einsum("hp,wq->whpq", wy, wx).reshape(W, H * PQ)
    pbg = np.ascontiguousarray(pbg.astype(BF16_NP))

    shared = {
        "pbg": pbg,
        "gp": gp,
        "kp": kp,
    }
    in_maps = []
    for b in range(B):
        m = dict(shared)
        m["v"] = np.ascontiguousarray(v[b].reshape(Cin, HW)).astype(BF16_NP)
        in_maps.append(m)
    return in_maps


def _run(inputs, **kwargs):
    nc = _get_nc()
    in_maps = _make_in_maps(
        np.asarray(inputs["v"]),
        np.asarray(inputs["k1"]),
        np.asarray(inputs["k2"]),
        np.asarray(inputs["conv_w"]),
        np.asarray(inputs["conv_b"]),
        np.asarray(inputs["bias"]),
    )
    res = run_bass_kernel_spmd(nc, in_maps, list(range(N_CORES)), **kwargs)
    out = np.stack(
        [res.results[b]["out"].reshape(Cout, H, W) for b in range(B)]
    ).astype(np.float32)
    return out, res


def kernel(**inputs):
    out, _ = _run(inputs)
    return out
